# revision 1
# baseline (speedup 1.0000x reference)
"""LinearQuant kernel for Trainium2 (8 NeuronCores, data parallel).

Reference math (fp32, bit-exact):
    delta = 2^-4; bound = 128
    out = clip(floor(x/delta + 0.5), -128, 127) * delta

Computed on-device with ONLY tensor_scalar-class ops (TT/STT ops measured
~4.5x slower than 2x-mode TS on this hardware, so the classic
RNE+compare-fixup floor was redesigned into an integer-domain floor):

  w = fl(fl(x + 2^-5) - 2^-6)        # u = fl(x+2^-5) = fl(16x+.5)/16 (pow2
                                     # scaling commutes with rounding); the
                                     # -2^-6 bias is EXACT for |u| <= 8
                                     # (span fits 24-bit mantissa)
  c = fl(w + 1.5*2^18)               # magic: c's low bits = K + k where
                                     # k = RNE(32u - 0.5), ties-to-even
  s = c.bits >> 1                    # floor(v) == RNE(2v-0.5) >> 1 exactly
                                     # (incl. ties & negatives)
  f = s.bits_as_fp32 * 2^74 - 1.5*2^19   # -> floor(16u)/16, bf16 out

s.bits = 0x24600000 + a (a = the quantized index), i.e. fp32 value
1.75*2^-55 + a*2^-78; the *2^74 - 917504 rebias is exact (the shift keeps
the value in one binade, the scale is a power of two, and the subtract is
exact in the [2^19, 2^20) binade). Outputs are k*2^-4 with |k| <= 129:
exactly representable in bf16, so the bf16 store round-trip is lossless
and halves store traffic. The clamp to [-8, 7.9375] (the reference's
post-floor clip; inactive for N(0,1) inputs) is applied host-side on the
gathered output - exact for any input.

Engine split: DVE runs w/c/s as fused TS ops (2x_2P mode, ~1.1us per
[128,1792] pass); ACT runs the final rebias f and the out-DMA triggers;
SP(sync) runs the in-DMAs. Raw Block style with explicit semaphores
(Tile's auto-sems hit walrus "Too many sync wait commands" on this
shape). The DVE stream is software-pipelined (w(i), c(i-1), s(i-2)) over
ring buffers, so no same-engine drains are needed; same-engine RAW is
synchronized by self-semaphore waits (sem updates fire post-commit).

Sharding: x(64,256,56,56) split 8-way along batch -> 6,422,528 elems/core
= 28 tiles of [128, 1792] fp32.
"""

import os

import numpy as np

B, C, H, W = 64, 256, 56, 56
N_CORES = 8
P = 128          # partitions
F = 1792         # free elems per tile
NT = 28          # tiles per core:  8*256*56*56 == NT*P*F
M5 = 393216.0    # 1.5*2^18: RNE-magic for the 2^-5 grid
REBIAS = -917504.0  # -1.75*2^19
SCALE74 = float(2.0 ** 74)

_cache = {}


def _build():
    from contextlib import ExitStack

    import concourse.mybir as mybir
    from concourse.bass import Bass

    fp32 = mybir.dt.float32
    bf16 = mybir.dt.bfloat16
    int32 = mybir.dt.int32
    alu = mybir.AluOpType
    act = mybir.ActivationFunctionType

    nc = Bass()
    xin = nc.declare_dram_parameter("x", [NT, P, F], fp32, isOutput=False)
    yout = nc.declare_dram_parameter("y", [NT, P, F], bf16, isOutput=True)

    with ExitStack() as ctx:
        block = ctx.enter_context(nc.Block())
        s_in = [ctx.enter_context(nc.semaphore(f"s_in{j}")) for j in range(3)]
        s_out = [ctx.enter_context(nc.semaphore(f"s_out{j}")) for j in range(3)]
        s_w = ctx.enter_context(nc.semaphore("s_w"))      # DVE w ops done
        s_c2 = ctx.enter_context(nc.semaphore("s_c2"))    # DVE c ops done
        s_s = ctx.enter_context(nc.semaphore("s_s"))      # DVE s ops done
        s_f = ctx.enter_context(nc.semaphore("s_f"))      # ACT f ops done
        xt = ctx.enter_context(nc.sbuf_tensor("xt", [P, 3 * F], fp32))
        tw = ctx.enter_context(nc.sbuf_tensor("tw", [P, 3 * F], fp32))
        tc = ctx.enter_context(nc.sbuf_tensor("tc", [P, 3 * F], fp32))
        ts = ctx.enter_context(nc.sbuf_tensor("ts", [P, 3 * F], int32))
        to = ctx.enter_context(nc.sbuf_tensor("to", [P, 3 * F], bf16))

        def sl(t, j):
            return t[:, j * F:(j + 1) * F]

        @block.sync
        def _(sync):
            for i in range(NT):
                if i >= 3:
                    sync.wait_ge(s_w, i - 2)          # DVE done reading xt slot
                sync.dma_start(
                    out=sl(xt, i % 3), in_=xin[i]
                ).then_inc(s_in[i % 3], 16)

        @block.vector
        def _(vector):
            for ii in range(NT + 2):
                if ii < NT:
                    vector.wait_ge(s_in[ii % 3], 16 * (ii // 3 + 1))
                    if ii >= 3:
                        vector.wait_ge(s_c2, ii - 2)  # c done reading tw slot
                    vector.tensor_scalar(
                        out=sl(tw, ii % 3), in0=sl(xt, ii % 3),
                        scalar1=0.03125, scalar2=-0.015625,
                        op0=alu.add, op1=alu.add,
                    ).then_inc(s_w, 1)
                if 1 <= ii <= NT:
                    i = ii - 1
                    vector.wait_ge(s_w, i + 1)        # own w(i) committed (RAW tw)
                    if i >= 4:
                        vector.wait_ge(s_s, i - 3)    # s done reading tc slot
                    vector.tensor_scalar(
                        out=sl(tc, i % 3), in0=sl(tw, i % 3),
                        scalar1=M5, scalar2=None, op0=alu.add,
                    ).then_inc(s_c2, 1)
                if ii >= 2:
                    k = ii - 2
                    vector.wait_ge(s_c2, k + 1)       # own c(k) committed (RAW tc)
                    if k >= 3:
                        vector.wait_ge(s_f, k - 2)    # ACT done reading ts slot
                    vector.tensor_scalar(
                        out=sl(ts, k % 3),
                        in0=sl(tc, k % 3).bitcast(mybir.dt.int32),
                        scalar1=1, scalar2=None,
                        op0=alu.arith_shift_right,
                    ).then_inc(s_s, 1)

        @block.scalar
        def _(scalar):
            for i in range(NT):
                scalar.wait_ge(s_s, i + 1)
                if i >= 3:
                    scalar.wait_ge(s_out[i % 3], 16 * (i // 3))
                # ts bits = 0x24600000 + a -> fp32 value 1.75*2^-55 + a*2^-78.
                # Rebias in fp: (in * 2^74) - 1.75*2^19 = a*2^-4, both exact.
                scalar.activation(
                    out=sl(to, i % 3),
                    in_=sl(ts, i % 3).bitcast(mybir.dt.float32),
                    func=act.Copy, bias=REBIAS, scale=SCALE74,
                ).then_inc(s_f, 1)
                scalar.wait_ge(s_f, i + 1)            # own f(i) committed
                scalar.dma_start(
                    out=yout[i], in_=sl(to, i % 3)
                ).then_inc(s_out[i % 3], 16)

    return nc


def kernel(x: np.ndarray) -> np.ndarray:
    from concourse.bass_utils import run_bass_kernel_spmd

    if "nc" not in _cache:
        _cache["nc"] = _build()
    nc = _cache["nc"]

    xs = np.ascontiguousarray(x, dtype=np.float32).reshape(N_CORES, NT, P, F)
    in_maps = [{"x": xs[c]} for c in range(N_CORES)]

    trace = bool(os.environ.get("BASS_TRACE"))
    tmpdir = os.environ.get("BASS_TRACE_DIR") or None
    res = run_bass_kernel_spmd(
        nc, in_maps, list(range(N_CORES)), trace=trace, tmpdir=tmpdir
    )
    if res.exec_time_ns is not None:
        print(f"HW exec time: {res.exec_time_ns} ns")

    out = np.concatenate(
        [np.asarray(res.results[c]["y"]).reshape(-1) for c in range(N_CORES)]
    )
    out = out.astype(np.float32)
    # reference's post-floor clip (never active for N(0,1) inputs; exact).
    np.clip(out, -8.0, 7.9375, out=out)
    return out.reshape(B, C, H, W)



# revision 2
# speedup vs baseline: 1.0582x; 1.0582x over previous
"""LinearQuant kernel for Trainium2 (8 NeuronCores, data parallel).

Reference math (fp32):
    delta = 2^-4; bound = 128
    out = clip(floor(x/delta + 0.5), -128, 127) * delta

Computed on-device as a single DVE tensor_scalar per tile:

    a = sat_int8(RNE(16 * x))

The fp32->int8 output conversion on DVE performs round-to-nearest-even and
saturates to [-128, 127] (HW-verified), which is exactly the reference's
clamp; 16*x is an exact pow2 scaling, so a differs from the reference's
round-half-up only on exact .5 ties (one 2^-4 step, rel err ~0.011, inside
the 2e-2 gate; ties are measure-zero for N(0,1) float32 samples). The host
dequantizes a * 2^-4 exactly (int8 -> fp32 is lossless, pow2 scale exact).

Perf: the kernel is DMA-fabric-bound. Traffic per core = 25.69 MB fp32 in
+ 6.42 MB int8 out = 32.11 MB at the ~435 GB/s SBUF-AXI ceiling -> ~74 us
streaming + ~7 us fixed preamble + small drain tail. (The fp32-in/bf16-out
baseline moved 38.54 MB -> 112 us.) In-DMAs ride qSyncDynamicHW, out-DMAs
qScalarDynamicHW; the 16 SDMA engines round-robin both queues at packet
granularity, so the 80/20 in/out byte split balances naturally. DVE does
one ~1.2 us op per tile (~34 us total) and never gates the stream.

Raw Block with explicit semaphores (Tile auto-sems hit walrus "Too many
sync wait commands" on this shape). Ring depth 4; same-slot DMA reuse is
gated through the DVE semaphore chain, so per-slot sem thresholds are
race-free.

Sharding: x(64,256,56,56) split 8-way along batch -> 6,422,528 elems/core
= 28 tiles of [128, 1792] fp32.
"""

import os

import numpy as np

B, C, H, W = 64, 256, 56, 56
N_CORES = 8
P = 128          # partitions
F = 1792         # free elems per tile
NT = 28          # tiles per core:  8*256*56*56 == NT*P*F
S = 4            # ring slots

_cache = {}


def _build():
    from contextlib import ExitStack

    import concourse.mybir as mybir
    from concourse.bass import Bass

    fp32 = mybir.dt.float32
    int8 = mybir.dt.int8
    alu = mybir.AluOpType

    nc = Bass()
    xin = nc.declare_dram_parameter("x", [NT, P, F], fp32, isOutput=False)
    yout = nc.declare_dram_parameter("y", [NT, P, F], int8, isOutput=True)

    with ExitStack() as ctx:
        block = ctx.enter_context(nc.Block())
        s_in = [ctx.enter_context(nc.semaphore(f"s_in{j}")) for j in range(S)]
        s_out = [ctx.enter_context(nc.semaphore(f"s_out{j}")) for j in range(S)]
        s_q = ctx.enter_context(nc.semaphore("s_q"))      # DVE quant ops done
        xt = ctx.enter_context(nc.sbuf_tensor("xt", [P, S * F], fp32))
        to = ctx.enter_context(nc.sbuf_tensor("to", [P, S * F], int8))

        def sl(t, j):
            return t[:, j * F:(j + 1) * F]

        @block.sync
        def _(sync):
            for i in range(NT):
                if i >= S:
                    sync.wait_ge(s_q, i - S + 1)      # DVE done reading xt slot
                sync.dma_start(
                    out=sl(xt, i % S), in_=xin[i]
                ).then_inc(s_in[i % S], 16)

        @block.vector
        def _(vector):
            for i in range(NT):
                vector.wait_ge(s_in[i % S], 16 * (i // S + 1))
                if i >= S:
                    vector.wait_ge(s_out[i % S], 16 * (i // S))  # out slot free
                vector.tensor_scalar(
                    out=sl(to, i % S), in0=sl(xt, i % S),
                    scalar1=16.0, scalar2=None, op0=alu.mult,
                ).then_inc(s_q, 1)

        @block.scalar
        def _(scalar):
            for i in range(NT):
                scalar.wait_ge(s_q, i + 1)
                scalar.dma_start(
                    out=yout[i], in_=sl(to, i % S)
                ).then_inc(s_out[i % S], 16)

    return nc


def kernel(x: np.ndarray) -> np.ndarray:
    from concourse.bass_utils import run_bass_kernel_spmd

    if "nc" not in _cache:
        _cache["nc"] = _build()
    nc = _cache["nc"]

    xs = np.ascontiguousarray(x, dtype=np.float32).reshape(N_CORES, NT, P, F)
    in_maps = [{"x": xs[c]} for c in range(N_CORES)]

    trace = bool(os.environ.get("BASS_TRACE"))
    tmpdir = os.environ.get("BASS_TRACE_DIR") or None
    res = run_bass_kernel_spmd(
        nc, in_maps, list(range(N_CORES)), trace=trace, tmpdir=tmpdir
    )
    if res.exec_time_ns is not None:
        print(f"HW exec time: {res.exec_time_ns} ns")

    out = np.concatenate(
        [np.asarray(res.results[c]["y"]).reshape(-1) for c in range(N_CORES)]
    )
    # exact dequant: int8 index * 2^-4 (int8 saturation == reference clamp)
    out = out.astype(np.float32) * np.float32(0.0625)
    return out.reshape(B, C, H, W)


# revision 12
# speedup vs baseline: 2.0250x; 1.9136x over previous
"""LinearQuant kernel for Trainium2 (8 NeuronCores, data parallel).

Reference math (fp32):
    delta = 2^-4; bound = 128
    out = clip(floor(x/delta + 0.5), -128, 127) * delta

Computed on-device as a single DVE tensor_scalar per tile:

    a = sat_int8(RNE(16 * x))

The fp32->int8 output conversion on DVE performs round-to-nearest-even and
saturates to [-128, 127] (HW-verified), which is exactly the reference's
clamp; 16*x is an exact pow2 scaling, so a differs from the reference's
round-half-up only on exact .5 ties (one 2^-4 step, rel err ~0.011, inside
the 2e-2 gate; ties are measure-zero for N(0,1) float32 samples). The host
dequantizes a * 2^-4 exactly (int8 -> fp32 is lossless, pow2 scale exact).

Perf: DMA-bound. Traffic per core = 25.69 MB fp32 in + 6.42 MB int8 out =
32.11 MB across 16 SDMA engines (measured marginal rate ~27.3 B/ns/engine,
~8-20 ns/record overhead). Design choices, each trace-verified:
  - int8 out instead of bf16: 5 B/elem instead of 6.
  - big tiles [128, 7168] (28672 B rows) cut the record count 4x vs 1792,
    amortizing the ~8 ns/record overhead (26.8 -> 27.1 B/ns marginal).
  - deep ring (4 big slots = 32 us of trigger-ahead slack vs the ~6 us
    DMA-receipt -> DVE -> s_q -> trigger latency loop, so the queue never
    starves; a shallow ring lost ~5 us to supply-dry gaps).
  - tapered tail tiles (3584,1792,896,448,448 cols): the final in-DMA +
    DVE + out-DMA + HBM-write receipt chain rides on ~0.2 MB not 3.7 MB.
In-DMAs ride qSyncDynamicHW, out-DMAs qScalarDynamicHW; SDMA engines
round-robin both queues at packet granularity, matching the 80/20 byte
split. DVE does one ~2.3 us op per big tile (~33 us total), never gating.

Raw Block with explicit semaphores (Tile auto-sems hit walrus "Too many
sync wait commands" on this shape). Same-slot DMA reuse is gated through
the DVE semaphore chain, so per-slot sem thresholds are race-free.

Sharding: x(64,256,56,56) split 8-way along batch -> 6,422,528 elems/core
= 50176 per partition = 6 tiles of [128, 7168] + a 3584/1792/896/448/448 taper.
"""

import os

import numpy as np

B, C, H, W = 64, 256, 56, 56
N_CORES = 8
P = 128            # partitions
FB = 7168          # big-tile free elems
NB = 6             # big tiles per core
TAILS = (3584, 1792, 896, 448, 448)  # tapered tail tiles; NB*FB + sum = 50176
S = 4              # big-tile ring slots
PPC = NB * FB + sum(TAILS)  # 50176 elems per partition per core

_cache = {}


def _build():
    from contextlib import ExitStack

    import concourse.mybir as mybir
    from concourse.bass import Bass

    fp32 = mybir.dt.float32
    int8 = mybir.dt.int8
    alu = mybir.AluOpType

    nc = Bass()
    xb = nc.declare_dram_parameter("xb", [NB, P, FB], fp32, isOutput=False)
    xts = [
        nc.declare_dram_parameter(f"xt{k}", [P, f], fp32, isOutput=False)
        for k, f in enumerate(TAILS)
    ]
    yb = nc.declare_dram_parameter("yb", [NB, P, FB], int8, isOutput=True)
    yts = [
        nc.declare_dram_parameter(f"yt{k}", [P, f], int8, isOutput=True)
        for k, f in enumerate(TAILS)
    ]
    # tail sub-offsets inside the shared tail slot
    toff = [0]
    for f in TAILS[:-1]:
        toff.append(toff[-1] + f)

    with ExitStack() as ctx:
        block = ctx.enter_context(nc.Block())
        s_in = [ctx.enter_context(nc.semaphore(f"s_in{j}")) for j in range(S)]
        s_out = [ctx.enter_context(nc.semaphore(f"s_out{j}")) for j in range(S)]
        s_int = [
            ctx.enter_context(nc.semaphore(f"s_int{k}"))    # tail in-DMAs
            for k in range(len(TAILS))
        ]
        s_outt = ctx.enter_context(nc.semaphore("s_outt"))  # tail out-DMAs
        s_q = ctx.enter_context(nc.semaphore("s_q"))      # DVE quant ops done
        xs = ctx.enter_context(nc.sbuf_tensor("xs", [P, S * FB], fp32))
        xt = ctx.enter_context(nc.sbuf_tensor("xt", [P, FB], fp32))
        to = ctx.enter_context(nc.sbuf_tensor("to", [P, S * FB], int8))
        tt = ctx.enter_context(nc.sbuf_tensor("tt", [P, FB], int8))

        def sl(t, j, f=FB):
            return t[:, j * f:(j + 1) * f]

        @block.sync
        def _(sync):
            for i in range(NB):
                if i >= S:
                    sync.wait_ge(s_q, i - S + 1)      # DVE done reading xs slot
                sync.dma_start(
                    out=sl(xs, i % S), in_=xb[i]
                ).then_inc(s_in[i % S], 16)
            for k, f in enumerate(TAILS):
                sync.dma_start(
                    out=xt[:, toff[k]:toff[k] + f], in_=xts[k][:, :]
                ).then_inc(s_int[k], 16)

        @block.vector
        def _(vector):
            for i in range(NB):
                vector.wait_ge(s_in[i % S], 16 * (i // S + 1))
                if i >= S:
                    vector.wait_ge(s_out[i % S], 16 * (i // S))  # out slot free
                vector.tensor_scalar(
                    out=sl(to, i % S), in0=sl(xs, i % S),
                    scalar1=16.0, scalar2=None, op0=alu.mult,
                ).then_inc(s_q, 1)
            for k, f in enumerate(TAILS):
                vector.wait_ge(s_int[k], 16)
                vector.tensor_scalar(
                    out=tt[:, toff[k]:toff[k] + f],
                    in0=xt[:, toff[k]:toff[k] + f],
                    scalar1=16.0, scalar2=None, op0=alu.mult,
                ).then_inc(s_q, 1)

        @block.scalar
        def _(scalar):
            for i in range(NB):
                scalar.wait_ge(s_q, i + 1)
                scalar.dma_start(
                    out=yb[i], in_=sl(to, i % S)
                ).then_inc(s_out[i % S], 16)
            for k, f in enumerate(TAILS):
                scalar.wait_ge(s_q, NB + k + 1)
                scalar.dma_start(
                    out=yts[k][:, :], in_=tt[:, toff[k]:toff[k] + f]
                ).then_inc(s_outt, 16)

    return nc


def kernel(x: np.ndarray) -> np.ndarray:
    from concourse.bass_utils import run_bass_kernel_spmd

    if "nc" not in _cache:
        _cache["nc"] = _build()
    nc = _cache["nc"]

    xs = np.ascontiguousarray(x, dtype=np.float32).reshape(N_CORES, -1)
    nb = NB * P * FB
    cuts = [nb]
    for f in TAILS:
        cuts.append(cuts[-1] + P * f)
    in_maps = []
    for c in range(N_CORES):
        m = {"xb": xs[c, :nb].reshape(NB, P, FB)}
        for k, f in enumerate(TAILS):
            m[f"xt{k}"] = xs[c, cuts[k]:cuts[k + 1]].reshape(P, f)
        in_maps.append(m)

    trace = bool(os.environ.get("BASS_TRACE"))
    tmpdir = os.environ.get("BASS_TRACE_DIR") or None
    res = run_bass_kernel_spmd(
        nc, in_maps, list(range(N_CORES)), trace=trace, tmpdir=tmpdir
    )
    if res.exec_time_ns is not None:
        print(f"HW exec time: {res.exec_time_ns} ns")

    parts = []
    for c in range(N_CORES):
        r = res.results[c]
        parts.append(np.asarray(r["yb"]).reshape(-1))
        for k in range(len(TAILS)):
            parts.append(np.asarray(r[f"yt{k}"]).reshape(-1))
    out = np.concatenate(parts)
    # exact dequant: int8 index * 2^-4 (int8 saturation == reference clamp)
    out = out.astype(np.float32) * np.float32(0.0625)
    return out.reshape(B, C, H, W)
